# revision 1
# baseline (speedup 1.0000x reference)
"""DeformConv3D on 8 TRN2 cores: H-sharded, dense 5-tap tent-weight gather.

Per core (h-band of 12 output rows + halos):
  P1: offset conv (27 taps, K=64 matmuls accumulated in PSUM) -> off scratch DRAM
  P2: trilinear gather as separable 5-tap tent-weighted sums on DVE
      (one (b,c) plane per partition; all shifts are AP offsets into a
       padded per-plane window; tent weights vanish outside the clamp range
       so padded reads are weight-zero)
  P3: main conv + bias -> output h-band
"""
import sys, os
import numpy as np
from contextlib import ExitStack

sys.path.insert(0, "/opt/trn_rl_repo")
from concourse import bass, bacc, tile, mybir
from concourse.bass_utils import run_bass_kernel_spmd

F32 = mybir.dt.float32
BF16 = mybir.dt.bfloat16
ALU = mybir.AluOpType
AF = mybir.ActivationFunctionType

B, C, L, H, W = 2, 64, 16, 96, 96
CO1, CO2 = 192, 64
NCORES = 8
HB = H // NCORES       # 12 output rows per core
HW_ROWS = 20           # x window rows per core: [12k-4, 12k+16)
HG = 14                # gather rows per core: [12k-1, 12k+13)
NPP = HG * W           # 1344 gather outputs per (plane, l)
ZPAD, XPAD = 20, 100   # gather window padded dims (taps +-2)
WIN = HW_ROWS * ZPAD * XPAD
CZP, CXP = 18, 98      # conv window padded dims (taps +-1)
TAPS = (-2, -1, 0, 1, 2)

_nc1_cache = None
_nc2_cache = None


def build_program1():
    nc = bacc.Bacc("TRN2", target_bir_lowering=False, debug=False, num_devices=NCORES)
    xwin = nc.dram_tensor("xwin", [B, C, L, HW_ROWS, W], F32, kind="ExternalInput").ap()
    w_off = nc.dram_tensor("w_off", [27, C, CO1], F32, kind="ExternalInput").ap()
    off_scr = nc.dram_tensor("off_band", [B, CO1, L, HG, W], F32, kind="ExternalOutput").ap()
    ctx = ExitStack()
    with tile.TileContext(nc) as tc:
        # ---------------- Phase 1: offset conv ----------------
        with tc.tile_pool(name="p1", bufs=1) as p1, \
             tc.tile_pool(name="p1ps", bufs=2, space="PSUM") as p1ps, \
             tc.tile_pool(name="p1o", bufs=3) as p1o:
            wofft = p1.tile([C, 27, CO1], F32)
            nc.sync.dma_start(wofft[:], w_off.rearrange("t c m -> c t m"))
            for b in range(B):
                xc = p1.tile([C, CZP, HW_ROWS, CXP], F32, tag="xc")
                nc.vector.memset(xc[:].rearrange("c z y x -> c (z y x)"), 0.0)
                for z in range(L):
                    nc.sync.dma_start(xc[:, 1 + z, :, 1:W + 1], xwin[b, :, z])
                for l in range(L):
                    for hc0, hcn in ((0, 5), (5, 5), (10, 4)):
                        nmm = hcn * W
                        for m0, mw in ((0, 128), (128, 64)):
                            ps = p1ps.tile([128, 480], F32, tag="ps1")
                            for t in range(27):
                                dz, rem = divmod(t, 9)
                                dy, dx = divmod(rem, 3)
                                rhs = xc[:, l + dz,
                                         2 + hc0 + dy:2 + hc0 + dy + hcn,
                                         dx:dx + W]
                                nc.tensor.matmul(
                                    ps[:mw, :nmm], wofft[:, t, m0:m0 + mw],
                                    rhs, start=(t == 0), stop=(t == 26))
                            ob = p1o.tile([128, 480], F32, tag="ob1")
                            nc.vector.tensor_copy(ob[:mw, :nmm], ps[:mw, :nmm])
                            nc.sync.dma_start(
                                off_scr[b, m0:m0 + mw, l, hc0:hc0 + hcn, :]
                                .rearrange("m h x -> m (h x)"),
                                ob[:mw, :nmm])

    nc.finalize()
    return nc


def build_program2():
    nc = bacc.Bacc("TRN2", target_bir_lowering=False, debug=False, num_devices=NCORES)
    xwin = nc.dram_tensor("xwin", [B, C, L, HW_ROWS, W], BF16, kind="ExternalInput").ap()
    w_conv = nc.dram_tensor("w_conv", [27, C, CO2], F32, kind="ExternalInput").ap()
    b_conv = nc.dram_tensor("b_conv", [CO2, 1], F32, kind="ExternalInput").ap()
    offs = nc.dram_tensor("offs", [128, 3, L, NPP], F32, kind="ExternalInput").ap()
    grids = nc.dram_tensor("grids", [128, 1, NPP], F32, kind="ExternalInput").ap()
    out_ext = nc.dram_tensor("out", [B, CO2, L, HB, W], F32, kind="ExternalOutput").ap()
    def_scr = nc.dram_tensor("def_scr", [B, C, L, HG, W], F32).ap()
    ctx = ExitStack()
    with tile.TileContext(nc) as tc:
        # ---------------- Phase 2: tent gather ----------------
        with tc.tile_pool(name="p2w", bufs=1) as p2w, \
             tc.tile_pool(name="p2", bufs=1) as p2:
            win = p2w.tile([128, HW_ROWS, ZPAD, XPAD], BF16)
            nc.vector.memset(win[:].rearrange("p y z x -> p (y z x)"), 0.0)
            for b in range(B):
                for z in range(L):
                    nc.sync.dma_start(
                        win[64 * b:64 * b + 64, :, 2 + z, 2:W + 2],
                        xwin[b, :, z])
            gr = p2w.tile([128, 1, NPP], F32)
            nc.sync.dma_start(gr[:], grids)
            zbias = p2w.tile([128, 1], F32)
            nc.vector.memset(zbias[:], 0.0)

            for l in range(L):
                offc = p2.tile([128, 3, NPP], F32, tag="off")
                nc.sync.dma_start(offc[:], offs[:, :, l, :])
                az = offc[:, 0]
                ay = offc[:, 1]
                ax = offc[:, 2]

                # tent weights lam[dim][tap] = relu(1 - |a - t|)  (bf16)
                tneg = p2.tile([128, NPP], F32, tag="tneg")
                tpos = p2.tile([128, NPP], F32, tag="tpos")

                def tents(a, dst_tag, taps):
                    row = []
                    for t in taps:
                        nc.vector.tensor_scalar(tpos[:], a, 1.0 - float(t), None, ALU.add)
                        nc.vector.tensor_scalar(tneg[:], a, -1.0, 1.0 + float(t), ALU.mult, ALU.add)
                        nc.vector.tensor_tensor(tpos[:], tpos[:], tneg[:], ALU.min)
                        lt = p2.tile([128, NPP], BF16, tag=f"{dst_tag}_{t}")
                        nc.scalar.activation(lt[:], tpos[:], AF.Relu, bias=zbias[:])
                        row.append(lt)
                    return row

                lamx = tents(ax, "lamx", TAPS)
                lamy = tents(ay, "lamy", TAPS)

                acc = p2.tile([128, NPP], F32, tag="acc")
                tmpi = p2.tile([128, NPP], F32, tag="tmpi")
                tmpb = p2.tile([128, NPP], F32, tag="tmpb")
                prod = p2.tile([128, NPP], BF16, tag="prod")
                lam = [None, lamy, lamx]
                for iz, sz in enumerate(TAPS):
                    lamz = tents(az, "lamz", (sz,))[0]
                    for iy, sy in enumerate(TAPS):
                        for ix, sx in enumerate(TAPS):
                            v = win[:, 3 + sy:3 + sy + HG,
                                    l + 2 + sz,
                                    2 + sx:2 + sx + W]
                            if ix == 0:
                                nc.vector.tensor_tensor(tmpi[:], lam[2][0][:], v, ALU.mult)
                            else:
                                nc.vector.tensor_tensor(prod[:], lam[2][ix][:], v, ALU.mult)
                                nc.vector.tensor_tensor(tmpi[:], tmpi[:], prod[:], ALU.add)
                        if iy == 0:
                            nc.vector.tensor_tensor(tmpb[:], lam[1][0][:], tmpi[:], ALU.mult)
                        else:
                            nc.vector.tensor_tensor(tmpi[:], lam[1][iy][:], tmpi[:], ALU.mult)
                            nc.vector.tensor_tensor(tmpb[:], tmpb[:], tmpi[:], ALU.add)
                    if iz == 0:
                        nc.vector.tensor_tensor(acc[:], lamz[:], tmpb[:], ALU.mult)
                    else:
                        nc.vector.tensor_tensor(tmpb[:], lamz[:], tmpb[:], ALU.mult)
                        nc.vector.tensor_tensor(acc[:], acc[:], tmpb[:], ALU.add)
                # zero rows whose global h is outside [0, 96)
                nc.vector.tensor_tensor(acc[:], acc[:], gr[:, 0], ALU.mult)
                for b in range(B):
                    nc.sync.dma_start(
                        def_scr[b, :, l].rearrange("c h x -> c (h x)"),
                        acc[64 * b:64 * b + 64, :])

        # ---------------- Phase 3: main conv ----------------
        with tc.tile_pool(name="p3", bufs=1) as p3, \
             tc.tile_pool(name="p3ps", bufs=2, space="PSUM") as p3ps, \
             tc.tile_pool(name="p3o", bufs=3) as p3o:
            wct = p3.tile([C, 27, CO2], F32)
            nc.sync.dma_start(wct[:], w_conv.rearrange("t c m -> c t m"))
            bct = p3.tile([CO2, 1], F32)
            nc.sync.dma_start(bct[:], b_conv)
            for b in range(B):
                dc = p3.tile([C, CZP, HG + 2, CXP], F32, tag="dc")
                nc.vector.memset(dc[:].rearrange("c z y x -> c (z y x)"), 0.0)
                for z in range(L):
                    nc.sync.dma_start(dc[:, 1 + z, 1:HG + 1, 1:W + 1], def_scr[b, :, z])
                for l in range(L):
                    for hc0, hcn in ((0, 5), (5, 5), (10, 2)):
                        nmm = hcn * W
                        ps = p3ps.tile([CO2, 480], F32, tag="ps3")
                        for t in range(27):
                            dz, rem = divmod(t, 9)
                            dy, dx = divmod(rem, 3)
                            # out row r=4+hc0+j -> dc y index r+dy-3
                            rhs = dc[:, l + dz,
                                     1 + hc0 + dy:1 + hc0 + dy + hcn,
                                     dx:dx + W]
                            nc.tensor.matmul(
                                ps[:, :nmm], wct[:, t, :],
                                rhs, start=(t == 0), stop=(t == 26))
                        ob = p3o.tile([CO2, 480], F32, tag="ob3")
                        nc.vector.tensor_scalar(ob[:, :nmm], ps[:, :nmm], bct[:], None, ALU.add)
                        nc.sync.dma_start(
                            out_ext[b, :, l, hc0:hc0 + hcn, :]
                            .rearrange("m h x -> m (h x)"),
                            ob[:, :nmm])
    nc.finalize()
    return nc


def kernel(x, w_off, w_conv, b_conv):
    global _nc1_cache, _nc2_cache
    x = np.asarray(x, dtype=np.float32)
    w_off = np.asarray(w_off, dtype=np.float32)
    w_conv = np.asarray(w_conv, dtype=np.float32)
    b_conv = np.asarray(b_conv, dtype=np.float32)

    if _nc1_cache is None:
        _nc1_cache = build_program1()
        _nc2_cache = build_program2()

    xp = np.zeros((B, C, L, H + 8, W), np.float32)
    xp[:, :, :, 4:4 + H, :] = x
    wofft = np.ascontiguousarray(
        w_off.reshape(CO1, C, 27).transpose(2, 1, 0))        # [27, C, CO1]
    wct = np.ascontiguousarray(
        w_conv.reshape(CO2, C, 27).transpose(2, 1, 0))       # [27, C, CO2]
    bc = np.ascontiguousarray(b_conv.reshape(CO2, 1))

    xwins = [np.ascontiguousarray(xp[:, :, :, 12 * k:12 * k + HW_ROWS, :])
             for k in range(NCORES)]
    import ml_dtypes
    xwins_bf = [w.astype(ml_dtypes.bfloat16) for w in xwins]
    in1 = [{"xwin": xwins[k], "w_off": wofft} for k in range(NCORES)]
    res1 = run_bass_kernel_spmd(_nc1_cache, in1, list(range(NCORES)))

    # reassemble full off field from per-core bands (band rows = 12k-1..12k+13)
    off_full = np.empty((B, CO1, L, H, W), np.float32)
    for k in range(NCORES):
        band = res1.results[k]["off_band"]
        off_full[:, :, :, 12 * k:12 * k + HB, :] = band[:, :, :, 1:1 + HB, :]
    # contiguous-view scramble: plane (b,c) offsets at spatial p, comp k =
    # flat element 3p+k of its 3-channel block. Per (l, h) row that is a
    # contiguous 288-float run, so a padded reshape + slice does it all.
    tri = off_full.reshape(B * C, L, 3 * H * W)
    trip = np.zeros((B * C, L, 3 * (H + 2) * W), np.float32)
    trip[:, :, 3 * W:3 * (H + 1) * W] = tri            # one pad row each side
    trip = trip.reshape(B * C, L, H + 2, W * 3)
    in2 = []
    gy = np.repeat(np.arange(HG, dtype=np.float32) + 3.0, W)
    gx = np.tile(np.arange(W, dtype=np.float32), HG)
    lgrid = np.arange(L, dtype=np.float32)[None, None, :, None]
    for k in range(NCORES):
        seg = trip[:, :, 12 * k:12 * k + HG, :]        # rows 12k-1..12k+13
        offs = np.ascontiguousarray(
            seg.reshape(128, L, HG * W, 3).transpose(0, 3, 1, 2))
        # displacements a = clamp(off + grid) - grid, computed on host
        hglobf = np.repeat(np.arange(HG, dtype=np.float32) + (12 * k - 1), W)
        offs[:, 0] = np.clip(offs[:, 0] + lgrid[0], 0.0, 15.0) - lgrid[0]
        offs[:, 1] = (np.clip(offs[:, 1] + hglobf[None, None, :], 0.0, 95.0)
                      - hglobf[None, None, :])
        offs[:, 2] = (np.clip(offs[:, 2] + gx[None, None, :], 0.0, 95.0)
                      - gx[None, None, :])
        hglob = np.repeat(np.arange(HG) + (12 * k - 1), W)
        ymask = ((hglob >= 0) & (hglob < H)).astype(np.float32)
        grids = np.broadcast_to(ymask[None, None], (128, 1, NPP)).copy()
        in2.append({
            "xwin": xwins_bf[k], "w_conv": wct, "b_conv": bc,
            "offs": offs,
            "grids": grids,
        })
    res2 = run_bass_kernel_spmd(_nc2_cache, in2, list(range(NCORES)))
    out = np.empty((B, CO2, L, H, W), np.float32)
    for k in range(NCORES):
        out[:, :, :, 12 * k:12 * k + HB, :] = res2.results[k]["out"]
    return out



# revision 2
# speedup vs baseline: 1.0056x; 1.0056x over previous
"""DeformConv3D on 8 TRN2 cores — fused SINGLE-LAUNCH kernel, L-sharded.

Core k owns view z-slices l in {2k, 2k+1} (all 128 (b,c) planes, full HxW):
  P1: offset conv for exactly the 12 "units" (ch, l2) = divmod(6k+j, 16),
      j in [0,6) x 2 batches, whose raw conv outputs interleave into the
      coords of view-l 2k/2k+1 (torch .view scramble is l-local!).
      K packs (c, z-tap-pair) = 128; writes bf16 slab scratch in DRAM.
  P2: de-interleave slab (stride-3) -> per-dim displacement f = clamp(off+g)-g
      -> separable 5-tap tent gather on DVE (bf16), ACT computes tents.
  P3: partial main conv: core k's deformed z-slices contribute to out
      l in [2k-1, 2k+2]; partials are output as-is (f32) and the HOST
      sums overlapping partials across cores and adds the bias.
No collectives, no cross-core traffic; one launch total.
"""
import sys
import numpy as np

sys.path.insert(0, "/opt/trn_rl_repo")
from concourse import bass, bacc, tile, mybir
from concourse.bass_utils import run_bass_kernel_spmd

F32 = mybir.dt.float32
BF16 = mybir.dt.bfloat16
ALU = mybir.AluOpType
AF = mybir.ActivationFunctionType

B, C, L, H, W = 2, 64, 16, 96, 96
HW = H * W
NCORES = 8
TAPS = (-2, -1, 0, 1, 2)
ROWCH = [(r, 5) for r in range(0, 95, 5)] + [(95, 1)]   # P1/P3 N-chunks
HC = 16                                                  # P2 rows per chunk
NHC = H // HC

_nc_cache = None


def build_program():
    nc = bacc.Bacc("TRN2", target_bir_lowering=False, debug=False,
                   num_devices=NCORES)
    x1 = nc.dram_tensor("x1", [B, 6, 3, C, 98, 98], BF16,
                        kind="ExternalInput").ap()
    x2 = nc.dram_tensor("x2", [128, 6, 100, 100], BF16,
                        kind="ExternalInput").ap()
    w1p = nc.dram_tensor("w1p", [6, 9, 128, 64], BF16,
                         kind="ExternalInput").ap()
    w1s = nc.dram_tensor("w1s", [6, 9, 64, 64], BF16,
                         kind="ExternalInput").ap()
    w3a = nc.dram_tensor("w3a", [9, 128, 64], BF16, kind="ExternalInput").ap()
    w3b = nc.dram_tensor("w3b", [9, 128, 64], BF16, kind="ExternalInput").ap()
    w3s2 = nc.dram_tensor("w3s2", [9, 64, 64], BF16, kind="ExternalInput").ap()
    w3s0 = nc.dram_tensor("w3s0", [9, 64, 64], BF16, kind="ExternalInput").ap()
    gy = nc.dram_tensor("gy", [128, HC * W], F32, kind="ExternalInput").ap()
    gx = nc.dram_tensor("gx", [128, HC * W], F32, kind="ExternalInput").ap()
    lb = nc.dram_tensor("lb", [128, 2], F32, kind="ExternalInput").ap()
    lbz = nc.dram_tensor("lbz", [128, 10], F32, kind="ExternalInput").ap()
    pout = nc.dram_tensor("pout", [8, 64, HW], F32, kind="ExternalOutput").ap()

    with tile.TileContext(nc) as tc:
        with tc.tile_pool(name="dram", bufs=1, space="DRAM") as dram:
            scratch = dram.tile([128, 6 * HW], BF16)
            defd = dram.tile([128, 2 * HW], BF16)

            # ---------------- P1: offset conv -> slab scratch ----------------
            with tc.tile_pool(name="p1w", bufs=1) as p1w, \
                 tc.tile_pool(name="p1x", bufs=2) as p1x, \
                 tc.tile_pool(name="p1ps", bufs=2, space="PSUM") as p1ps, \
                 tc.tile_pool(name="p1o", bufs=3) as p1o:
                w1pt = p1w.tile([128, 6, 9, 64], BF16)
                nc.sync.dma_start(w1pt[:], w1p.rearrange("j g p m -> p j g m"))
                w1st = p1w.tile([64, 6, 9, 64], BF16)
                nc.sync.dma_start(w1st[:], w1s.rearrange("j g p m -> p j g m"))
                for b in range(B):
                    for j in range(6):
                        xu = p1x.tile([128, 3, 98, 98], BF16, tag="xu")
                        nc.sync.dma_start(
                            xu[0:64], x1[b, j, 0:3].rearrange("z c y x -> c z y x"))
                        nc.sync.dma_start(
                            xu[64:128, 0:2],
                            x1[b, j, 1:3].rearrange("z c y x -> c z y x"))
                        for (r0, rn) in ROWCH:
                            n = rn * W
                            ps = p1ps.tile([64, 480], F32, tag="ps1")
                            g = 0
                            for dy in range(3):
                                for dx in range(3):
                                    nc.tensor.matmul(
                                        ps[:, :n], w1pt[:, j, g, :],
                                        xu[:, 0, dy + r0:dy + r0 + rn, dx:dx + W],
                                        start=(g == 0), stop=False)
                                    g += 1
                            g = 0
                            for dy in range(3):
                                for dx in range(3):
                                    nc.tensor.matmul(
                                        ps[:, :n], w1st[:, j, g, :],
                                        xu[0:64, 2, dy + r0:dy + r0 + rn, dx:dx + W],
                                        start=False, stop=(g == 8))
                                    g += 1
                            ob = p1o.tile([64, 480], BF16, tag="ob1")
                            nc.vector.tensor_copy(ob[:, :n], ps[:, :n])
                            nc.sync.dma_start(
                                scratch[b * 64:(b + 1) * 64,
                                        j * HW + r0 * W: j * HW + r0 * W + n],
                                ob[:, :n])

            # ---------------- P2: tent gather -> defd ----------------
            with tc.tile_pool(name="p2c", bufs=1) as p2c, \
                 tc.tile_pool(name="p2win", bufs=2) as p2win, \
                 tc.tile_pool(name="p2sl", bufs=2) as p2sl, \
                 tc.tile_pool(name="p2k", bufs=1) as p2k:
                gyt = p2c.tile([128, HC * W], F32)
                nc.sync.dma_start(gyt[:], gy)
                gxt = p2c.tile([128, HC * W], F32)
                nc.sync.dma_start(gxt[:], gx)
                lbt = p2c.tile([128, 2], F32)
                nc.sync.dma_start(lbt[:], lb)
                lbzt = p2c.tile([128, 10], F32)
                nc.sync.dma_start(lbzt[:], lbz)
                cb = p2c.tile([128, 3], F32)
                nc.vector.memset(cb[:, 0:1], -2.0)
                nc.vector.memset(cb[:, 1:2], -1.0)
                nc.vector.memset(cb[:, 2:3], 2.0)
                # bias AP for value -t (t in TAPS): +2,+1,0,-1,-2
                bias_of = {-2: cb[:, 2:3], -1: 1.0, 0: 0.0,
                           1: cb[:, 1:2], 2: cb[:, 0:1]}
                neg1 = cb[:, 1:2]

                NP = HC * W   # 1536
                for hc in range(NHC):
                    h0 = HC * hc
                    win = p2win.tile([128, 6, 20, 100], BF16, tag="win")
                    nc.sync.dma_start(win[:], x2[:, :, h0:h0 + 20, :])
                    for lv in range(2):
                        slab = p2sl.tile([128, 3 * NP], BF16, tag="slab")
                        nc.sync.dma_start(
                            slab[:], scratch[:, 3 * lv * HW + hc * 3 * NP:
                                             3 * lv * HW + (hc + 1) * 3 * NP])
                        sv = slab[:].rearrange("p (n k) -> p k n", k=3)
                        offz = p2k.tile([128, NP], F32, tag="offz")
                        offy = p2k.tile([128, NP], F32, tag="offy")
                        offx = p2k.tile([128, NP], F32, tag="offx")
                        nc.vector.tensor_copy(offz[:], sv[:, 0])
                        nc.vector.tensor_copy(offy[:], sv[:, 1])
                        nc.vector.tensor_copy(offx[:], sv[:, 2])
                        # s_z = clamp(off_z + l, 0, 15) = min(Relu(off_z + l), 15)
                        nc.scalar.activation(offz[:], offz[:], AF.Relu,
                                             bias=lbt[:, lv:lv + 1])
                        nc.vector.tensor_scalar(offz[:], offz[:], 15.0, None,
                                                ALU.min)
                        # f_y = clamp(off_y + gy + h0, 0, 95) - h0 - gy
                        nc.vector.tensor_tensor(offy[:], offy[:], gyt[:], ALU.add)
                        nc.vector.tensor_scalar(offy[:], offy[:], float(h0), 0.0,
                                                ALU.add, ALU.max)
                        nc.vector.tensor_scalar(offy[:], offy[:], 95.0, float(h0),
                                                ALU.min, ALU.subtract)
                        nc.vector.tensor_tensor(offy[:], offy[:], gyt[:],
                                                ALU.subtract)
                        # f_x = clamp(off_x + gx, 0, 95) - gx
                        nc.vector.tensor_tensor(offx[:], offx[:], gxt[:], ALU.add)
                        nc.vector.tensor_scalar(offx[:], offx[:], 0.0, 95.0,
                                                ALU.max, ALU.min)
                        nc.vector.tensor_tensor(offx[:], offx[:], gxt[:],
                                                ALU.subtract)
                        # tents
                        u = p2k.tile([128, NP], F32, tag="u")
                        lamx, lamy = [], []
                        for t in TAPS:
                            nc.scalar.activation(u[:], offx[:], AF.Abs,
                                                 bias=bias_of[t])
                            lt = p2k.tile([128, NP], BF16, tag=f"lamx{t}")
                            nc.scalar.activation(lt[:], u[:], AF.Relu,
                                                 bias=1.0, scale=neg1)
                            lamx.append(lt)
                        for t in TAPS:
                            nc.scalar.activation(u[:], offy[:], AF.Abs,
                                                 bias=bias_of[t])
                            lt = p2k.tile([128, NP], BF16, tag=f"lamy{t}")
                            nc.scalar.activation(lt[:], u[:], AF.Relu,
                                                 bias=1.0, scale=neg1)
                            lamy.append(lt)
                        lamz = p2k.tile([128, NP], BF16, tag="lamz")
                        tmpi = p2k.tile([128, NP], BF16, tag="tmpi")
                        prod = p2k.tile([128, NP], BF16, tag="prod")
                        tmpb = p2k.tile([128, NP], BF16, tag="tmpb")
                        tmpz = p2k.tile([128, NP], F32, tag="tmpz")
                        acc = p2k.tile([128, NP], F32, tag="acc")
                        accb = p2k.tile([128, NP], BF16, tag="accb")
                        for iz, sz in enumerate(TAPS):
                            # u = |s_z - (l + sz)|, bias column = -(2k+lv+sz)
                            nc.scalar.activation(
                                u[:], offz[:], AF.Abs,
                                bias=lbzt[:, lv * 5 + iz:lv * 5 + iz + 1])
                            nc.scalar.activation(lamz[:], u[:], AF.Relu,
                                                 bias=1.0, scale=neg1)
                            zi = lv + sz + 2
                            for iy, sy in enumerate(TAPS):
                                for ix, sx in enumerate(TAPS):
                                    v = win[:, zi, sy + 2:sy + 2 + HC,
                                            sx + 2:sx + 2 + W]
                                    if ix == 0:
                                        nc.vector.tensor_tensor(
                                            tmpi[:], lamx[0][:], v, ALU.mult)
                                    else:
                                        nc.vector.tensor_tensor(
                                            prod[:], lamx[ix][:], v, ALU.mult)
                                        nc.vector.tensor_tensor(
                                            tmpi[:], tmpi[:], prod[:], ALU.add)
                                if iy == 0:
                                    nc.vector.tensor_tensor(
                                        tmpb[:], lamy[0][:], tmpi[:], ALU.mult)
                                else:
                                    nc.vector.tensor_tensor(
                                        prod[:], lamy[iy][:], tmpi[:], ALU.mult)
                                    nc.vector.tensor_tensor(
                                        tmpb[:], tmpb[:], prod[:], ALU.add)
                            if iz == 0:
                                nc.vector.tensor_tensor(
                                    acc[:], lamz[:], tmpb[:], ALU.mult)
                            elif iz < 4:
                                nc.vector.tensor_tensor(
                                    tmpz[:], lamz[:], tmpb[:], ALU.mult)
                                nc.vector.tensor_tensor(
                                    acc[:], acc[:], tmpz[:], ALU.add)
                            else:
                                nc.vector.tensor_tensor(
                                    tmpz[:], lamz[:], tmpb[:], ALU.mult)
                                nc.vector.tensor_tensor(
                                    accb[:], acc[:], tmpz[:], ALU.add)
                        nc.sync.dma_start(
                            defd[:, lv * HW + h0 * W: lv * HW + h0 * W + NP],
                            accb[:])

            # ---------------- P3: partial main conv -> pout ----------------
            with tc.tile_pool(name="p3w", bufs=1) as p3w, \
                 tc.tile_pool(name="p3ps", bufs=2, space="PSUM") as p3ps, \
                 tc.tile_pool(name="p3o", bufs=4) as p3o:
                w3at = p3w.tile([128, 9, 64], BF16)
                nc.sync.dma_start(w3at[:], w3a.rearrange("g p m -> p g m"))
                w3bt = p3w.tile([128, 9, 64], BF16)
                nc.sync.dma_start(w3bt[:], w3b.rearrange("g p m -> p g m"))
                w3s2t = p3w.tile([64, 9, 64], BF16)
                nc.sync.dma_start(w3s2t[:], w3s2.rearrange("g p m -> p g m"))
                w3s0t = p3w.tile([64, 9, 64], BF16)
                nc.sync.dma_start(w3s0t[:], w3s0.rearrange("g p m -> p g m"))
                for b in range(B):
                    dcA = p3w.tile([128, 98, 98], BF16, tag=f"dcA{b}")
                    nc.vector.memset(dcA[:].rearrange("p y x -> p (y x)"), 0.0)
                    dcB = p3w.tile([64, 98, 98], BF16, tag=f"dcB{b}")
                    nc.vector.memset(dcB[:].rearrange("p y x -> p (y x)"), 0.0)
                    nc.sync.dma_start(
                        dcA[0:64, 1:97, 1:97],
                        defd[b * 64:(b + 1) * 64, 0:HW]
                        .rearrange("c (y x) -> c y x", y=96))
                    nc.sync.dma_start(
                        dcA[64:128, 1:97, 1:97],
                        defd[b * 64:(b + 1) * 64, HW:2 * HW]
                        .rearrange("c (y x) -> c y x", y=96))
                    nc.sync.dma_start(
                        dcB[0:64, 1:97, 1:97],
                        defd[b * 64:(b + 1) * 64, HW:2 * HW]
                        .rearrange("c (y x) -> c y x", y=96))
                    for (r0, rn) in ROWCH:
                        n = rn * W
                        # group 0: out l=2k-1 (dz=2 from z=2k = dcA lower)
                        # group 1: out l=2k   (dz=1 lower, dz=2 upper)
                        # group 2: out l=2k+1 (dz=0 lower, dz=1 upper)
                        # group 3: out l=2k+2 (dz=0 from z=2k+1 = dcB)
                        specs = [(w3s2t, dcA[0:64], 64),
                                 (w3at, dcA[:], 128),
                                 (w3bt, dcA[:], 128),
                                 (w3s0t, dcB[0:64], 64)]
                        for li, (wt, dct, kk) in enumerate(specs):
                            ps = p3ps.tile([64, 480], F32, tag=f"ps3{li % 2}")
                            g = 0
                            for dy in range(3):
                                for dx in range(3):
                                    nc.tensor.matmul(
                                        ps[:, :n], wt[:, g, :],
                                        dct[:, dy + r0:dy + r0 + rn, dx:dx + W],
                                        start=(g == 0), stop=(g == 8))
                                    g += 1
                            ob = p3o.tile([64, 480], F32, tag=f"ob3{li % 2}")
                            nc.vector.tensor_copy(ob[:, :n], ps[:, :n])
                            nc.sync.dma_start(
                                pout[li * 2 + b, :, r0 * W: r0 * W + n],
                                ob[:, :n])
    nc.finalize()
    return nc


def kernel(x, w_off, w_conv, b_conv):
    global _nc_cache
    import ml_dtypes
    x = np.asarray(x, dtype=np.float32)
    w_off = np.asarray(w_off, dtype=np.float32)
    w_conv = np.asarray(w_conv, dtype=np.float32)
    b_conv = np.asarray(b_conv, dtype=np.float32)

    if _nc_cache is None:
        _nc_cache = build_program()

    bf = ml_dtypes.bfloat16
    # P1 source: pad z/y/x by 1
    xp1 = np.zeros((B, C, L + 2, 98, 98), bf)
    xp1[:, :, 1:L + 1, 1:H + 1, 1:W + 1] = x.astype(bf)
    # P2 source: pad z/y/x by 2
    xp2 = np.zeros((B, C, L + 4, 100, 100), bf)
    xp2[:, :, 2:L + 2, 2:H + 2, 2:W + 2] = x.astype(bf)

    woff_r = w_off.reshape(64, 3, C, 3, 3, 3)      # [m', ch, c, dz, dy, dx]
    wt_off = np.ascontiguousarray(
        np.transpose(woff_r, (1, 4, 5, 3, 2, 0)))  # [ch, dy, dx, dz, c, m']
    wc_t = np.transpose(w_conv, (3, 4, 2, 1, 0))   # [dy, dx, dz, c, m]
    w3a = np.ascontiguousarray(
        wc_t[:, :, 1:3].reshape(9, 128, 64)).astype(bf)
    w3b = np.ascontiguousarray(
        wc_t[:, :, 0:2].reshape(9, 128, 64)).astype(bf)
    w3s2 = np.ascontiguousarray(wc_t[:, :, 2].reshape(9, 64, 64)).astype(bf)
    w3s0 = np.ascontiguousarray(wc_t[:, :, 0].reshape(9, 64, 64)).astype(bf)

    gyt = np.broadcast_to(
        np.repeat(np.arange(HC, dtype=np.float32), W)[None], (128, HC * W))
    gxt = np.broadcast_to(
        np.tile(np.arange(W, dtype=np.float32), HC)[None], (128, HC * W))
    gyt = np.ascontiguousarray(gyt)
    gxt = np.ascontiguousarray(gxt)

    in_maps = []
    for k in range(NCORES):
        units = [divmod(6 * k + j, 16) for j in range(6)]   # (ch, l2)
        x1 = np.empty((B, 6, 3, C, 98, 98), bf)
        for j, (ch, l2) in enumerate(units):
            x1[:, j] = np.transpose(xp1[:, :, l2:l2 + 3], (0, 2, 1, 3, 4))
        x2 = np.ascontiguousarray(
            xp2[:, :, 2 * k:2 * k + 6].reshape(128, 6, 100, 100))
        w1p = np.empty((6, 9, 128, 64), bf)
        w1s = np.empty((6, 9, 64, 64), bf)
        for j, (ch, l2) in enumerate(units):
            w1p[j] = wt_off[ch, :, :, 0:2].reshape(9, 128, 64)
            w1s[j] = wt_off[ch, :, :, 2].reshape(9, 64, 64)
        lbv = np.array([2 * k, 2 * k + 1], np.float32)
        lb = np.broadcast_to(lbv[None], (128, 2)).copy()
        # lbz[:, lv*5+iz] = -(2k + lv + sz), sz = TAPS[iz]
        lbzv = np.array([-(2 * k + lv + sz) for lv in range(2) for sz in TAPS],
                        np.float32)
        lbz = np.broadcast_to(lbzv[None], (128, 10)).copy()
        in_maps.append({
            "x1": x1, "x2": x2, "w1p": w1p, "w1s": w1s,
            "w3a": w3a, "w3b": w3b, "w3s2": w3s2, "w3s0": w3s0,
            "gy": gyt, "gx": gxt, "lb": lb, "lbz": lbz,
        })

    res = run_bass_kernel_spmd(_nc_cache, in_maps, list(range(NCORES)))

    out = np.zeros((B, 64, L, H, W), np.float32)
    for k in range(NCORES):
        po = res.results[k]["pout"]        # [8, 64, HW]
        for li in range(4):
            lg = 2 * k - 1 + li
            if 0 <= lg < L:
                for b in range(B):
                    out[b, :, lg] += po[li * 2 + b].reshape(64, H, W)
    out += b_conv[None, :, None, None, None]
    return out
